# revision 8
# baseline (speedup 1.0000x reference)
"""Chamfer loss on 8 TRN2 NeuronCores.

Strategy (data-parallel over batch):
  - B=8 batches -> one batch per core.
  - Host prep per batch: augment points so a single K=5 matmul emits the
    full squared-distance tile directly:
        xa = [x0; x1; x2; |x|^2; 1]    (5, N)  -- stationary (lhsT)
        ya = [-2*y0; -2*y1; -2*y2; 1; |y|^2]  (5, M)  -- moving (rhs)
        d2[i, j] = sum_k xa[k, i] * ya[k, j]
  - On core: sweep the 8192 x 8192 distance matrix in [128 x 2048] PSUM
    supertiles (4 matmuls each).  One fused DVE tensor_tensor_reduce per
    supertile does the fp32 PSUM read once: out = max(d2, 0) cast to a
    bf16 SBUF stage (the reference relu; commutes with min), accum_out =
    running row-min.  The col-min runs as bf16 tensor_tensor min (2x DVE
    mode) into a bf16 column accumulator.  Loop is y-supertile-outer so
    each finished colacc slice is transposed (TensorE) and
    partition-min-reduced while the next slice is swept.  Partition sums
    via a ones-vector matmul.
  - Output per core: [1, 2] = (sum of relu'd row minima, sum of relu'd
    col minima). Host: loss = sum over cores of both / (B * N).
"""

import sys

for _p in ("/opt/trn_rl_repo", "/root/.axon_site/_ro/trn_rl_repo"):
    if _p not in sys.path:
        sys.path.insert(0, _p)

import numpy as np

B = 8
N = 8192          # x points per batch
M = 8192          # y points per batch
P = 128           # partition tile (x-chunk size)
NCHUNK = N // P   # 64
SUPER = 2048      # y supertile width
NSUPER = M // SUPER  # 4

_COMPILED = {}


def _build(reps: int = 1):
    import concourse.bass as bass
    import concourse.bacc as bacc
    import concourse.mybir as mybir
    import concourse.tile as tile

    f32 = mybir.dt.float32
    bf16 = mybir.dt.bfloat16
    AX = mybir.AxisListType
    OP = mybir.AluOpType

    nc = bacc.Bacc("TRN2", target_bir_lowering=False, debug=False, num_devices=B)

    xa_d = nc.dram_tensor("xa", [5, N], f32, kind="ExternalInput")
    ya_d = nc.dram_tensor("ya", [5, M], f32, kind="ExternalInput")
    id_d = nc.dram_tensor("ident", [P, P], f32, kind="ExternalInput")
    out_d = nc.dram_tensor("out", [1, 2], f32, kind="ExternalOutput")

    with tile.TileContext(nc) as tc:
        with (
            tc.tile_pool(name="persist", bufs=1) as pp,
            tc.tile_pool(name="stage", bufs=3) as sp,
        ):
            xa = pp.tile([5, N], f32)
            ya = pp.tile([5, M], f32)
            identf = pp.tile([P, P], f32)
            ident = pp.tile([P, P], bf16)
            ones = pp.tile([P, 1], f32)
            colacc = pp.tile([P, M], bf16)
            rowpart = pp.tile([P, NCHUNK * NSUPER], f32)
            rowmins = pp.tile([P, NCHUNK], f32)
            colmins = pp.tile([P, M // P], f32)
            sums = pp.tile([1, 2], f32)

            nc.sync.dma_start(xa[:], xa_d[:])
            nc.sync.dma_start(ya[:], ya_d[:])
            nc.sync.dma_start(identf[:], id_d[:])
            nc.vector.tensor_copy(ident[:], identf[:])
            nc.vector.memset(ones[:], 1.0)

            with tc.tile_pool(name="psum_main", bufs=2, space="PSUM") as pm:
                for _rep in range(reps):
                    for s in range(NSUPER):
                        cslice = colacc[:, s * SUPER:(s + 1) * SUPER]
                        for c in range(NCHUNK):
                            lhs = xa[:, c * P:(c + 1) * P]
                            ps = pm.tile([P, SUPER], f32, tag="ps")
                            for t in range(SUPER // 512):
                                y0 = s * SUPER + t * 512
                                nc.tensor.matmul(
                                    ps[:, t * 512:(t + 1) * 512],
                                    lhs,
                                    ya[:, y0:y0 + 512],
                                )
                            k = c * NSUPER + s
                            # ACT: stage the supertile to SBUF as bf16
                            dst = cslice if c == 0 else sp.tile(
                                [P, SUPER], bf16, tag="stg"
                            )
                            nc.scalar.copy(dst, ps[:])
                            # DVE: row-min off the bf16 stage (4x mode)
                            nc.vector.tensor_reduce(
                                rowpart[:, k:k + 1], dst, axis=AX.X, op=OP.min
                            )
                            # DVE: col-min update in bf16 (2x mode)
                            if c != 0:
                                nc.vector.tensor_tensor(
                                    cslice, dst, cslice, op=OP.min
                                )
                        # transpose finished colacc slice; min over partitions
                        nblk = SUPER // P  # 16 blocks of 128
                        pst = pm.tile([P, SUPER], bf16, tag="ps")
                        for kb in range(nblk):
                            blk = s * nblk + kb
                            nc.tensor.transpose(
                                pst[:, kb * P:(kb + 1) * P],
                                colacc[:, blk * P:(blk + 1) * P],
                                ident[:],
                            )
                        nc.vector.tensor_reduce(
                            colmins[:, s * nblk:(s + 1) * nblk],
                            pst[:].rearrange("p (k f) -> p k f", f=P),
                            axis=AX.X,
                            op=OP.min,
                        )

                # ---- row minima -> per-chunk mins, then relu ----
                nc.vector.tensor_reduce(
                    rowmins[:],
                    rowpart[:].rearrange("p (c s) -> p c s", s=NSUPER),
                    axis=AX.X,
                    op=OP.min,
                )
                nc.vector.tensor_scalar_max(rowmins[:], rowmins[:], 0.0)
                nc.vector.tensor_scalar_max(colmins[:], colmins[:], 0.0)

            # ---- partition sums via ones-matmul, then free-dim sums ----
            with tc.tile_pool(name="psum_epi", bufs=1, space="PSUM") as pe:
                fin = pe.tile([1, 2 * NCHUNK], f32, tag="fin")
                nc.tensor.matmul(fin[:, 0:NCHUNK], ones[:], rowmins[:])
                nc.tensor.matmul(
                    fin[:, NCHUNK:NCHUNK + M // P], ones[:], colmins[:]
                )
                nc.vector.tensor_reduce(
                    sums[:, 0:1], fin[:, 0:NCHUNK], axis=AX.X, op=OP.add
                )
                nc.vector.tensor_reduce(
                    sums[:, 1:2], fin[:, NCHUNK:NCHUNK + M // P],
                    axis=AX.X, op=OP.add,
                )
                nc.sync.dma_start(out_d[:], sums[:])

    nc.compile()
    return nc


def _get_nc():
    if "nc" not in _COMPILED:
        _COMPILED["nc"] = _build()
    return _COMPILED["nc"]


def _prep_inputs(x: np.ndarray, y: np.ndarray):
    """Per-core input maps from full inputs."""
    x = np.asarray(x, dtype=np.float32)
    y = np.asarray(y, dtype=np.float32)
    ident = np.eye(P, dtype=np.float32)
    in_maps = []
    for b in range(B):
        xb, yb = x[b], y[b]  # [N, 3]
        xa = np.empty((5, N), dtype=np.float32)
        xa[0:3] = xb.T
        xa[3] = (xb * xb).sum(axis=1)
        xa[4] = 1.0
        ya = np.empty((5, M), dtype=np.float32)
        ya[0:3] = -2.0 * yb.T
        ya[3] = 1.0
        ya[4] = (yb * yb).sum(axis=1)
        in_maps.append({"xa": xa, "ya": ya, "ident": ident})
    return in_maps


def kernel(x: np.ndarray, y: np.ndarray) -> np.ndarray:
    from concourse.bass_utils import run_bass_kernel_spmd

    nc = _get_nc()
    in_maps = _prep_inputs(x, y)
    res = run_bass_kernel_spmd(nc, in_maps, list(range(B)))
    total = 0.0
    for b in range(B):
        o = res.results[b]["out"]
        total += float(o[0, 0]) + float(o[0, 1])
    loss = total / (B * N)
    return np.float32(loss)
